# revision 21
# baseline (speedup 1.0000x reference)
"""Trainium2 Bass kernel for nn_ActorCritic (value MLP + per-sample hypernetwork).

Sharding: pure data parallel. Batch 4096 split as 512 samples per core across
8 NeuronCores; the small value-network weights are replicated (host
pre-transposed/packed so the device never transposes anything).

Per-core work:
  - Value net (TensorE, bf16): X^T [256,512] -> W1^T matmuls -> ELU -> W2^T
    -> ELU -> w3 -> value [512]. Kept in [feature, batch] layout so biases are
    per-partition ACT scalars. ELU(z) = relu(z) + exp(min(z,0)) - 1; the -1
    is folded into the *next* layer's bias on the host (b' = b - W.sum(axis=1)),
    and the relu/exp branches are kept as separate bf16 tiles -- the next
    layer's matmul runs over both and accumulates in PSUM (linearity), so no
    elementwise add is ever needed.
  - Hypernetwork (VectorE): option rows hold per-sample MLP weights
    (64x256, 64, 64x64, 64, 32x64, 32 packed), stored bf16. With samples on
    partitions, a custom DVE op computes a running dot product
    scan(ADD, Src0*Src1) over the weight stream against a stride-0-broadcast
    activation vector; segment sums are extracted by strided subtraction of
    the prefix sums (GpSimd does the small fix-up ops).
"""

import numpy as np

P = 128
B_LOCAL = 512
NBLK = B_LOCAL // P  # 4 blocks of 128 samples
NCORES = 8
Z = 256
H = 1024
OPT_W = 22688
OUT_W = 33

# hypernet packing offsets (HyperOption z=256->64->64->32)
OFF_W1, OFF_B1 = 0, 16384
OFF_W2, OFF_B2 = 16448, 20544
OFF_W3, OFF_B3 = 20608, 22656

# device-side repacked option stream: per layer, segment o = [w_o | b_o], so a
# prefix-sum difference over one segment directly yields w_o . x + b_o.
# L1: 64 segments of 257; L2: 64 of 65; L3: 32 of 65.
RE_L1, RE_L2, RE_L3 = 0, 64 * 257, 64 * 257 + 64 * 65


def _option_perm():
    idx = []
    for o in range(64):
        idx.extend(range(OFF_W1 + o * 256, OFF_W1 + (o + 1) * 256))
        idx.append(OFF_B1 + o)
    for o in range(64):
        idx.extend(range(OFF_W2 + o * 64, OFF_W2 + (o + 1) * 64))
        idx.append(OFF_B2 + o)
    for o in range(32):
        idx.extend(range(OFF_W3 + o * 64, OFF_W3 + (o + 1) * 64))
        idx.append(OFF_B3 + o)
    assert len(idx) == OPT_W
    return np.asarray(idx, dtype=np.int64)

# packed bf16 value-net matmul operands: xt | w1t | w2t | w3t
WPACK_W = 2 * 512 + 2 * 1024 + 8 * 1024 + 8  # 11272
# packed f32 biases: b1 | -b1 | b2' | -b2' | b3'
BPACK_W = 8 + 8 + 8 + 8 + 1  # 33

_NC_CACHE = {}


def _bf16(a):
    import ml_dtypes

    return np.asarray(a, dtype=ml_dtypes.bfloat16)


def _register_mul_scan():
    """Register (once) a custom DVE op: out[p,k] = sum_{j<=k} in0[p,j]*in1[p,j]."""
    from concourse.dve_spec import Spec, Src0, Src1, AluOp, scan, lower
    from concourse import dve_ops
    from concourse.dve_uop import DveOpSpec

    name = "MUL_SCAN_ANT"
    for op in dve_ops.OPS:
        if op.name == name:
            return op

    def _ref(in0, in1, s0, s1, imm2):
        p = in0.shape[0]
        prod = in0.astype(np.float32).reshape(p, -1) * in1.astype(
            np.float32
        ).reshape(p, -1)
        return np.cumsum(prod, axis=-1).reshape(in0.shape).astype(np.float32)

    spec = Spec(body=scan(AluOp.ADD, Src0 * Src1), reference=_ref)
    row = dve_ops._CUSTOM_DVE_ROW_BASE + len(dve_ops.OPS)
    assert row < 0x20
    dve_ops._SUB_OPCODE_FOR_NAME[name] = row
    shas = {}
    for ver in ("v3", "v4"):
        tmp = DveOpSpec(name=name, opcode=row, uops=lower(spec, ver=ver), rd1_en=True)
        shas[ver] = tmp.sha(ver)
    op = dve_ops.DveOp(name, spec, subdim=False, uops_sha=shas)
    dve_ops.OPS.append(op)
    dve_ops.CUSTOM_DVE_SPECS[name] = spec
    return op


def _build_nc():
    from contextlib import ExitStack
    from concourse import bacc, bass, tile, mybir

    MUL_SCAN = _register_mul_scan()
    AF = mybir.ActivationFunctionType
    f32 = mybir.dt.float32
    bf16 = mybir.dt.bfloat16

    nc = bacc.Bacc("TRN2", target_bir_lowering=False, debug=False)

    opt_d = nc.declare_dram_parameter("option", [B_LOCAL, OPT_W], bf16, isOutput=False)
    x_d = nc.declare_dram_parameter("inputs", [B_LOCAL, Z], f32, isOutput=False)
    wpack_d = nc.declare_dram_parameter("wpack", [P, WPACK_W], bf16, isOutput=False)
    bpack_d = nc.declare_dram_parameter("bpack", [P, BPACK_W], f32, isOutput=False)
    out_d = nc.declare_dram_parameter("out", [B_LOCAL, OUT_W], f32, isOutput=True)

    with tile.TileContext(nc) as tc, ExitStack() as ctx:
        wpool = ctx.enter_context(tc.tile_pool(name="weights", bufs=1))
        optp = ctx.enter_context(tc.tile_pool(name="opt", bufs=3))
        scanp = ctx.enter_context(tc.tile_pool(name="scan", bufs=2))
        xblk = ctx.enter_context(tc.tile_pool(name="xblk", bufs=2))
        hp = ctx.enter_context(tc.tile_pool(name="hyper", bufs=2))
        vp = ctx.enter_context(tc.tile_pool(name="vnet", bufs=2))
        outp = ctx.enter_context(tc.tile_pool(name="outst", bufs=4))
        psum = ctx.enter_context(
            tc.tile_pool(name="psum", bufs=6, space=bass.MemorySpace.PSUM)
        )
        psv = ctx.enter_context(
            tc.tile_pool(name="psv", bufs=2, space=bass.MemorySpace.PSUM)
        )

        # ---- replicated matmul operands (single packed bf16 DMA) --------
        wp_sb = wpool.tile([P, WPACK_W], bf16)
        nc.scalar.dma_start(wp_sb[:], wpack_d[:])
        o = 0
        xt_sb = wp_sb[:, o : o + 2 * B_LOCAL].rearrange("p (k b) -> p k b", k=2)
        o += 2 * B_LOCAL
        w1_sb = wp_sb[:, o : o + 2 * H].rearrange("p (k h) -> p k h", k=2)
        o += 2 * H
        w2_sb = wp_sb[:, o : o + 8 * H].rearrange("p (k h) -> p k h", k=8)
        o += 8 * H
        w3_sb = wp_sb[:, o : o + 8]
        o += 8
        assert o == WPACK_W

        bp_sb = wpool.tile([P, BPACK_W], f32)
        nc.scalar.dma_start(bp_sb[:], bpack_d[:])
        b1_sb = bp_sb[:, 0:8]
        nb1_sb = bp_sb[:, 8:16]
        b2_sb = bp_sb[:, 16:24]
        nb2_sb = bp_sb[:, 24:32]
        b3r_sb = bp_sb[:, 32:33]

        out_tiles = [
            outp.tile([P, OUT_W], f32, tag="outst", name=f"out_st{g}")
            for g in range(NBLK)
        ]

        # ---- hypernetwork (VectorE scans), per 128-sample block --------
        # option is host-repacked so segment o = [w_o | b_o]; with the
        # activation vector extended by a trailing 1.0, a prefix-sum diff
        # over one segment yields w_o . x + b_o directly.
        def seg_ends(st, n_seg, seg):
            # st cols: 0 = zero pad, 1..n*seg = prefix sums
            v = st[:, 1 : 1 + n_seg * seg].rearrange("p (o i) -> p o i", i=seg)
            return v[:, :, seg - 1 : seg].squeeze(2)

        def seg_starts(st, n_seg, seg):
            v = st[:, 0 : n_seg * seg].rearrange("p (o i) -> p o i", i=seg)
            return v[:, :, 0:1].squeeze(2)

        for g in range(NBLK):
            rows = slice(g * P, (g + 1) * P)
            xb = xblk.tile([P, Z + 1], f32, tag="xb")
            nc.sync.dma_start(xb[:, 0:Z], x_d[rows, :])
            nc.vector.memset(xb[:, Z : Z + 1], 1.0)

            # layer 1: 64 segments of 257, split into 2 scans of 32
            h1p = hp.tile([P, 64], f32, tag="h1p")
            for sc in range(2):
                w = 32 * 257  # 8224
                ot = optp.tile([P, w], bf16, tag="opt")
                nc.sync.dma_start(ot[:], opt_d[rows, sc * w : (sc + 1) * w])
                st = scanp.tile([P, 1 + w], f32, tag="scan")
                nc.vector.memset(st[:, 0:1], 0.0)
                nc.vector._custom_dve(
                    MUL_SCAN,
                    out=st[:, 1 : 1 + w],
                    in0=ot[:],
                    in1=xb[:].unsqueeze(1).broadcast_to([P, 32, Z + 1]),
                )
                nc.vector.tensor_sub(
                    h1p[:, sc * 32 : (sc + 1) * 32],
                    seg_ends(st, 32, 257),
                    seg_starts(st, 32, 257),
                )
            h1 = hp.tile([P, 65], f32, tag="h1")
            nc.scalar.activation(h1[:, 0:64], h1p[:], AF.Relu)
            nc.vector.memset(h1[:, 64:65], 1.0)

            # layer 2: 64 segments of 65
            w2l = 64 * 65  # 4160
            ot2 = optp.tile([P, w2l], bf16, tag="opt")
            nc.sync.dma_start(ot2[:], opt_d[rows, RE_L2 : RE_L2 + w2l])
            st2 = scanp.tile([P, 1 + w2l], f32, tag="scan")
            nc.vector.memset(st2[:, 0:1], 0.0)
            nc.vector._custom_dve(
                MUL_SCAN,
                out=st2[:, 1 : 1 + w2l],
                in0=ot2[:],
                in1=h1[:].unsqueeze(1).broadcast_to([P, 64, 65]),
            )
            h2p = hp.tile([P, 64], f32, tag="h2p")
            nc.vector.tensor_sub(h2p[:], seg_ends(st2, 64, 65), seg_starts(st2, 64, 65))
            h2 = hp.tile([P, 65], f32, tag="h2")
            nc.scalar.activation(h2[:, 0:64], h2p[:], AF.Relu)
            nc.vector.memset(h2[:, 64:65], 1.0)

            # layer 3: 32 segments of 65, no relu; diff lands in the output tile
            w3l = 32 * 65  # 2080
            ot3 = optp.tile([P, w2l], bf16, tag="opt")
            nc.sync.dma_start(ot3[:, 0:w3l], opt_d[rows, RE_L3 : RE_L3 + w3l])
            st3 = scanp.tile([P, 1 + w2l], f32, tag="scan")
            nc.vector.memset(st3[:, 0:1], 0.0)
            nc.vector._custom_dve(
                MUL_SCAN,
                out=st3[:, 1 : 1 + w3l],
                in0=ot3[:, 0:w3l],
                in1=h2[:].unsqueeze(1).broadcast_to([P, 32, 65]),
            )
            nc.vector.tensor_sub(
                out_tiles[g][:, 1:33], seg_ends(st3, 32, 65), seg_starts(st3, 32, 65)
            )

        # ---- value network (TensorE bf16), all 512 samples at once -----
        # ELU+1 kept as two bf16 tiles r (relu branch) and e (exp branch);
        # the next layer's matmuls run over both and accumulate in PSUM.
        r1_sb = vp.tile([P, 8, B_LOCAL], bf16, tag="r1v", bufs=1)
        e1_sb = vp.tile([P, 8, B_LOCAL], bf16, tag="e1v", bufs=1)
        for mt in range(8):
            ps = psum.tile([P, B_LOCAL], f32, tag="ps")
            for kt in range(2):
                nc.tensor.matmul(
                    ps[:],
                    w1_sb[:, kt, mt * P : (mt + 1) * P],
                    xt_sb[:, kt, :],
                    start=(kt == 0),
                    stop=(kt == 1),
                )
            nc.scalar.activation(
                r1_sb[:, mt, :], ps[:], AF.Relu, bias=b1_sb[:, mt : mt + 1]
            )
            u = vp.tile([P, B_LOCAL], f32, tag="elu_u")
            nc.scalar.activation(
                u[:], ps[:], AF.Relu, bias=nb1_sb[:, mt : mt + 1], scale=-1.0
            )
            nc.scalar.activation(e1_sb[:, mt, :], u[:], AF.Exp, scale=-1.0)

        r2_sb = vp.tile([P, 8, B_LOCAL], bf16, tag="r2v", bufs=1)
        e2_sb = vp.tile([P, 8, B_LOCAL], bf16, tag="e2v", bufs=1)
        for mt in range(8):
            ps = psum.tile([P, B_LOCAL], f32, tag="ps")
            n_acc = 16
            i = 0
            for kt in range(8):
                for h1v in (r1_sb, e1_sb):
                    nc.tensor.matmul(
                        ps[:],
                        w2_sb[:, kt, mt * P : (mt + 1) * P],
                        h1v[:, kt, :],
                        start=(i == 0),
                        stop=(i == n_acc - 1),
                    )
                    i += 1
            nc.scalar.activation(
                r2_sb[:, mt, :], ps[:], AF.Relu, bias=b2_sb[:, mt : mt + 1]
            )
            u = vp.tile([P, B_LOCAL], f32, tag="elu_u")
            nc.scalar.activation(
                u[:], ps[:], AF.Relu, bias=nb2_sb[:, mt : mt + 1], scale=-1.0
            )
            nc.scalar.activation(e2_sb[:, mt, :], u[:], AF.Exp, scale=-1.0)

        for g in range(NBLK):
            pv = psv.tile([P, 1], f32, tag="pv")
            n_acc = 16
            i = 0
            for kt in range(8):
                for h2v in (r2_sb, e2_sb):
                    nc.tensor.matmul(
                        pv[:],
                        h2v[:, kt, g * P : (g + 1) * P],
                        w3_sb[:, kt : kt + 1],
                        start=(i == 0),
                        stop=(i == n_acc - 1),
                    )
                    i += 1
            nc.scalar.activation(
                out_tiles[g][:, 0:1], pv[:], AF.Identity, bias=b3r_sb[:, 0:1]
            )

        for g in range(NBLK):
            rows = slice(g * P, (g + 1) * P)
            nc.scalar.dma_start(out_d[rows, :], out_tiles[g][:])

    nc.compile()
    return nc


def _get_nc():
    if "nc" not in _NC_CACHE:
        _NC_CACHE["nc"] = _build_nc()
    return _NC_CACHE["nc"]


def _prep_in_maps(inputs):
    x = np.ascontiguousarray(np.asarray(inputs["inputs"], dtype=np.float32))
    opt = np.asarray(inputs["option"], dtype=np.float32)
    w1 = np.asarray(inputs["w1"], dtype=np.float32)
    b1 = np.asarray(inputs["b1"], dtype=np.float32)
    w2 = np.asarray(inputs["w2"], dtype=np.float32)
    b2 = np.asarray(inputs["b2"], dtype=np.float32)
    w3 = np.asarray(inputs["w3"], dtype=np.float32)
    b3 = np.asarray(inputs["b3"], dtype=np.float32)

    opt_bf = _bf16(opt[:, _option_perm()])

    # weight [K, M] with K across 128-partition tiles -> [128, n_k_tiles * M]
    def ktiled(a):  # a: [K, M]
        k, m = a.shape
        return a.reshape(k // P, P, m).transpose(1, 0, 2).reshape(P, -1)

    w1t = ktiled(w1.T)  # [128, 2*1024]
    w2t = ktiled(w2.T)  # [128, 8*1024]
    w3t = w3.reshape(8, P).T  # [128, 8]
    b1t = b1.reshape(8, P).T  # [128, 8]
    # device computes elu+1 (= relu(z)+exp(min(z,0))); fold the -1 into the
    # consumer's bias: b' = b - W.sum(axis=1)
    b2p = b2 - w2.sum(axis=1)
    b2t = b2p.reshape(8, P).T
    b3p = float(b3[0] - w3.sum())
    b3r = np.full((P, 1), b3p, dtype=np.float32)
    wtail = np.concatenate([w1t, w2t, w3t], axis=1)
    bpack = np.ascontiguousarray(
        np.concatenate([b1t, -b1t, b2t, -b2t, b3r], axis=1), dtype=np.float32
    )
    assert bpack.shape == (P, BPACK_W)

    in_maps = []
    for c in range(NCORES):
        sl = slice(c * B_LOCAL, (c + 1) * B_LOCAL)
        xs = np.ascontiguousarray(x[sl])
        xt = ktiled(xs.T)  # [128, 2*512]
        wpack = np.ascontiguousarray(_bf16(np.concatenate([xt, wtail], axis=1)))
        assert wpack.shape == (P, WPACK_W)
        in_maps.append(
            {
                "option": np.ascontiguousarray(opt_bf[sl]),
                "inputs": xs,
                "wpack": wpack,
                "bpack": bpack,
            }
        )
    return in_maps


def _ensure_ntff_hook():
    """Provide antenv.axon_hooks (missing in this image) so trace=True works."""
    import sys
    import types

    if "antenv.axon_hooks" in sys.modules:
        return
    mod = types.ModuleType("antenv.axon_hooks")
    state = {"hook": None}
    mod.set_axon_ntff_profile_hook = lambda h: state.__setitem__("hook", h)
    mod.get_axon_ntff_profile_hook = lambda: state["hook"]
    sys.modules["antenv.axon_hooks"] = mod
    import antenv

    antenv.axon_hooks = mod
    try:
        from trn_agent_boot.trn_boot import _ntff_profile_via_ctypes

        hook = _ntff_profile_via_ctypes("/opt/axon/libaxon_pjrt.so")
        mod.set_axon_ntff_profile_hook(hook)
    except Exception as e:  # degrade: tracing skipped, run still works
        print(f"ntff hook setup failed: {e}")


def run(inputs, trace=False):
    """Returns (full_output [4096, 33] float32, exec_time_ns or None)."""
    from concourse.bass_utils import run_bass_kernel_spmd

    if trace:
        _ensure_ntff_hook()
    nc = _get_nc()
    in_maps = _prep_in_maps(inputs)
    res = run_bass_kernel_spmd(nc, in_maps, core_ids=list(range(NCORES)), trace=trace)
    out = np.concatenate([res.results[i]["out"] for i in range(NCORES)], axis=0)
    return out.astype(np.float32), res.exec_time_ns


def kernel(**inputs):
    out, _ = run(inputs, trace=False)
    return out


# revision 26
# speedup vs baseline: 1.0996x; 1.0996x over previous
"""Trainium2 Bass kernel for nn_ActorCritic (value MLP + per-sample hypernetwork).

Sharding: pure data parallel. Batch 4096 split as 512 samples per core across
8 NeuronCores; the small value-network weights are replicated (host
pre-transposed/packed so the device never transposes anything).

Per-core work:
  - Value net (TensorE, bf16): X^T [256,512] -> W1^T matmuls -> ELU -> W2^T
    -> ELU -> w3 -> value [512]. Kept in [feature, batch] layout so biases are
    per-partition ACT scalars. ELU(z) = relu(z) + exp(min(z,0)) - 1; the -1
    is folded into the *next* layer's bias on the host (b' = b - W.sum(axis=1)),
    and the relu/exp branches are kept as separate bf16 tiles -- the next
    layer's matmul runs over both and accumulates in PSUM (linearity), so no
    elementwise add is ever needed.
  - Hypernetwork (VectorE): option rows hold per-sample MLP weights
    (64x256, 64, 64x64, 64, 32x64, 32 packed), stored bf16. With samples on
    partitions, a custom DVE op computes a running dot product
    scan(ADD, Src0*Src1) over the weight stream against a stride-0-broadcast
    activation vector; segment sums are extracted by strided subtraction of
    the prefix sums (GpSimd does the small fix-up ops).
"""

import numpy as np

P = 128
B_LOCAL = 512
NBLK = B_LOCAL // P  # 4 blocks of 128 samples
NCORES = 8
Z = 256
H = 1024
OPT_W = 22688
OUT_W = 33

# hypernet packing offsets (HyperOption z=256->64->64->32)
OFF_W1, OFF_B1 = 0, 16384
OFF_W2, OFF_B2 = 16448, 20544
OFF_W3, OFF_B3 = 20608, 22656

# device-side repacked option stream: per layer, segment o = [w_o | b_o], so a
# prefix-sum difference over one segment directly yields w_o . x + b_o.
# L1: 64 segments of 257; L2: 64 of 65; L3: 32 of 65.
RE_L1, RE_L2, RE_L3 = 0, 64 * 257, 64 * 257 + 64 * 65


def _option_perm():
    idx = []
    for o in range(64):
        idx.extend(range(OFF_W1 + o * 256, OFF_W1 + (o + 1) * 256))
        idx.append(OFF_B1 + o)
    for o in range(64):
        idx.extend(range(OFF_W2 + o * 64, OFF_W2 + (o + 1) * 64))
        idx.append(OFF_B2 + o)
    for o in range(32):
        idx.extend(range(OFF_W3 + o * 64, OFF_W3 + (o + 1) * 64))
        idx.append(OFF_B3 + o)
    assert len(idx) == OPT_W
    return np.asarray(idx, dtype=np.int64)

# packed bf16 value-net matmul operands: xt | w1t | w2t | w3t
WPACK_W = 2 * 512 + 2 * 1024 + 8 * 1024 + 8  # 11272
# packed f32 biases: b1 | -b1 | b2' | -b2' | b3'
BPACK_W = 8 + 8 + 8 + 8 + 1  # 33

_NC_CACHE = {}


def _bf16(a):
    import ml_dtypes

    return np.asarray(a, dtype=ml_dtypes.bfloat16)


def _register_op(name, make_spec):
    """Register (once) a custom DVE op by name; returns the DveOp."""
    from concourse.dve_spec import lower
    from concourse import dve_ops
    from concourse.dve_uop import DveOpSpec

    for op in dve_ops.OPS:
        if op.name == name:
            return op
    spec = make_spec()
    row = dve_ops._CUSTOM_DVE_ROW_BASE + len(dve_ops.OPS)
    assert row < 0x20
    dve_ops._SUB_OPCODE_FOR_NAME[name] = row
    shas = {}
    for ver in ("v3", "v4"):
        tmp = DveOpSpec(name=name, opcode=row, uops=lower(spec, ver=ver), rd1_en=True)
        shas[ver] = tmp.sha(ver)
    op = dve_ops.DveOp(name, spec, subdim=False, uops_sha=shas)
    dve_ops.OPS.append(op)
    dve_ops.CUSTOM_DVE_SPECS[name] = spec
    return op


def _register_mul_scan():
    """out[p,k] = sum_{j<=k} in0[p,j]*in1[p,j] (fp32 running dot product)."""
    from concourse.dve_spec import Spec, Src0, Src1, AluOp, scan

    def mk():
        def _ref(in0, in1, s0, s1, imm2):
            p = in0.shape[0]
            prod = in0.astype(np.float32).reshape(p, -1) * in1.astype(
                np.float32
            ).reshape(p, -1)
            return np.cumsum(prod, axis=-1).reshape(in0.shape).astype(np.float32)

        return Spec(body=scan(AluOp.ADD, Src0 * Src1), reference=_ref)

    return _register_op("MUL_SCAN_ANT", mk)


def _register_sub_relu():
    """out = relu(in0 - in1)."""
    from concourse.dve_spec import Spec, Src0, Src1, relu

    def mk():
        return Spec(
            body=relu(Src0 - Src1),
            reference=lambda in0, in1, s0, s1, imm2: np.maximum(
                in0.astype(np.float32) - in1.astype(np.float32), 0.0
            ),
        )

    return _register_op("SUB_RELU_ANT", mk)


def _build_nc():
    from contextlib import ExitStack
    from concourse import bacc, bass, tile, mybir

    MUL_SCAN = _register_mul_scan()
    SUB_RELU = _register_sub_relu()
    AF = mybir.ActivationFunctionType
    f32 = mybir.dt.float32
    bf16 = mybir.dt.bfloat16

    nc = bacc.Bacc("TRN2", target_bir_lowering=False, debug=False)

    opt_d = nc.declare_dram_parameter("option", [B_LOCAL, OPT_W], bf16, isOutput=False)
    x_d = nc.declare_dram_parameter("inputs", [B_LOCAL, Z], f32, isOutput=False)
    wpack_d = nc.declare_dram_parameter("wpack", [P, WPACK_W], bf16, isOutput=False)
    bpack_d = nc.declare_dram_parameter("bpack", [P, BPACK_W], f32, isOutput=False)
    out_d = nc.declare_dram_parameter("out", [B_LOCAL, OUT_W], f32, isOutput=True)

    with tile.TileContext(nc) as tc, ExitStack() as ctx:
        wpool = ctx.enter_context(tc.tile_pool(name="weights", bufs=1))
        optp = ctx.enter_context(tc.tile_pool(name="opt", bufs=3))
        scanp = ctx.enter_context(tc.tile_pool(name="scan", bufs=2))
        xblk = ctx.enter_context(tc.tile_pool(name="xblk", bufs=2))
        hp = ctx.enter_context(tc.tile_pool(name="hyper", bufs=2))
        vp = ctx.enter_context(tc.tile_pool(name="vnet", bufs=2))
        outp = ctx.enter_context(tc.tile_pool(name="outst", bufs=4))
        psum = ctx.enter_context(
            tc.tile_pool(name="psum", bufs=6, space=bass.MemorySpace.PSUM)
        )
        psv = ctx.enter_context(
            tc.tile_pool(name="psv", bufs=2, space=bass.MemorySpace.PSUM)
        )

        out_tiles = [
            outp.tile([P, OUT_W], f32, tag="outst", name=f"out_st{g}")
            for g in range(NBLK)
        ]

        # ---- block-0 hypernet input DMAs first: they gate the DVE scan
        # pipeline, so they go ahead of the (PE-only) weight pack in the
        # sync-ring FIFO.
        L1W = 32 * 257  # 8224
        xb0 = xblk.tile([P, Z + 1], f32, tag="xb")
        nc.sync.dma_start(xb0[:, 0:Z], x_d[0:P, :])
        nc.vector.memset(xb0[:, Z : Z + 1], 1.0)
        ot0 = []
        for sc in range(2):
            ot = optp.tile([P, L1W], bf16, tag="opt", name=f"ot0_{sc}")
            nc.sync.dma_start(ot[:], opt_d[0:P, sc * L1W : (sc + 1) * L1W])
            ot0.append(ot)

        # ---- replicated matmul operands (single packed bf16 DMA) --------
        wp_sb = wpool.tile([P, WPACK_W], bf16)
        nc.sync.dma_start(wp_sb[:], wpack_d[:])
        o = 0
        xt_sb = wp_sb[:, o : o + 2 * B_LOCAL].rearrange("p (k b) -> p k b", k=2)
        o += 2 * B_LOCAL
        w1_sb = wp_sb[:, o : o + 2 * H].rearrange("p (k h) -> p k h", k=2)
        o += 2 * H
        w2_sb = wp_sb[:, o : o + 8 * H].rearrange("p (k h) -> p k h", k=8)
        o += 8 * H
        w3_sb = wp_sb[:, o : o + 8]
        o += 8
        assert o == WPACK_W

        bp_sb = wpool.tile([P, BPACK_W], f32)
        nc.scalar.dma_start(bp_sb[:], bpack_d[:])
        b1_sb = bp_sb[:, 0:8]
        nb1_sb = bp_sb[:, 8:16]
        b2_sb = bp_sb[:, 16:24]
        nb2_sb = bp_sb[:, 24:32]
        b3r_sb = bp_sb[:, 32:33]

        # ---- hypernetwork (VectorE scans), per 128-sample block --------
        # option is host-repacked so segment o = [w_o | b_o]; with the
        # activation vector extended by a trailing 1.0, a prefix-sum diff
        # over one segment yields w_o . x + b_o directly.
        def seg_ends(st, n_seg, seg):
            # st cols: 0 = zero pad, 1..n*seg = prefix sums
            v = st[:, 1 : 1 + n_seg * seg].rearrange("p (o i) -> p o i", i=seg)
            return v[:, :, seg - 1 : seg].squeeze(2)

        def seg_starts(st, n_seg, seg):
            v = st[:, 0 : n_seg * seg].rearrange("p (o i) -> p o i", i=seg)
            return v[:, :, 0:1].squeeze(2)

        for g in range(NBLK):
            rows = slice(g * P, (g + 1) * P)
            if g == 0:
                xb = xb0
            else:
                xb = xblk.tile([P, Z + 1], f32, tag="xb")
                nc.sync.dma_start(xb[:, 0:Z], x_d[rows, :])
                nc.vector.memset(xb[:, Z : Z + 1], 1.0)

            # layer 1: 64 segments of 257, split into 2 scans of 32
            h1 = hp.tile([P, 65], f32, tag="h1")
            for sc in range(2):
                if g == 0:
                    ot = ot0[sc]
                else:
                    ot = optp.tile([P, L1W], bf16, tag="opt")
                    nc.sync.dma_start(ot[:], opt_d[rows, sc * L1W : (sc + 1) * L1W])
                st = scanp.tile([P, 1 + L1W], f32, tag="scan")
                nc.vector.memset(st[:, 0:1], 0.0)
                nc.vector._custom_dve(
                    MUL_SCAN,
                    out=st[:, 1 : 1 + L1W],
                    in0=ot[:],
                    in1=xb[:].unsqueeze(1).broadcast_to([P, 32, Z + 1]),
                )
                nc.vector._custom_dve(
                    SUB_RELU,
                    out=h1[:, sc * 32 : (sc + 1) * 32],
                    in0=seg_ends(st, 32, 257),
                    in1=seg_starts(st, 32, 257),
                )
            nc.vector.memset(h1[:, 64:65], 1.0)

            # layer 2: 64 segments of 65
            w2l = 64 * 65  # 4160
            ot2 = optp.tile([P, w2l], bf16, tag="opt")
            nc.sync.dma_start(ot2[:], opt_d[rows, RE_L2 : RE_L2 + w2l])
            st2 = scanp.tile([P, 1 + w2l], f32, tag="scan")
            nc.vector.memset(st2[:, 0:1], 0.0)
            nc.vector._custom_dve(
                MUL_SCAN,
                out=st2[:, 1 : 1 + w2l],
                in0=ot2[:],
                in1=h1[:].unsqueeze(1).broadcast_to([P, 64, 65]),
            )
            h2 = hp.tile([P, 65], f32, tag="h2")
            nc.vector._custom_dve(
                SUB_RELU,
                out=h2[:, 0:64],
                in0=seg_ends(st2, 64, 65),
                in1=seg_starts(st2, 64, 65),
            )
            nc.vector.memset(h2[:, 64:65], 1.0)

            # layer 3: 32 segments of 65, no relu; diff lands in the output tile
            w3l = 32 * 65  # 2080
            ot3 = optp.tile([P, w2l], bf16, tag="opt")
            nc.sync.dma_start(ot3[:, 0:w3l], opt_d[rows, RE_L3 : RE_L3 + w3l])
            st3 = scanp.tile([P, 1 + w2l], f32, tag="scan")
            nc.vector.memset(st3[:, 0:1], 0.0)
            nc.vector._custom_dve(
                MUL_SCAN,
                out=st3[:, 1 : 1 + w3l],
                in0=ot3[:, 0:w3l],
                in1=h2[:].unsqueeze(1).broadcast_to([P, 32, 65]),
            )
            nc.vector.tensor_sub(
                out_tiles[g][:, 1:33], seg_ends(st3, 32, 65), seg_starts(st3, 32, 65)
            )

        # ---- value network (TensorE bf16), all 512 samples at once -----
        # ELU+1 kept as two bf16 tiles r (relu branch) and e (exp branch);
        # the next layer's matmuls run over both and accumulate in PSUM.
        r1_sb = vp.tile([P, 8, B_LOCAL], bf16, tag="r1v", bufs=1)
        e1_sb = vp.tile([P, 8, B_LOCAL], bf16, tag="e1v", bufs=1)
        for mt in range(8):
            ps = psum.tile([P, B_LOCAL], f32, tag="ps")
            for kt in range(2):
                nc.tensor.matmul(
                    ps[:],
                    w1_sb[:, kt, mt * P : (mt + 1) * P],
                    xt_sb[:, kt, :],
                    start=(kt == 0),
                    stop=(kt == 1),
                )
            nc.scalar.activation(
                r1_sb[:, mt, :], ps[:], AF.Relu, bias=b1_sb[:, mt : mt + 1]
            )
            u = vp.tile([P, B_LOCAL], f32, tag="elu_u")
            nc.scalar.activation(
                u[:], ps[:], AF.Relu, bias=nb1_sb[:, mt : mt + 1], scale=-1.0
            )
            nc.scalar.activation(e1_sb[:, mt, :], u[:], AF.Exp, scale=-1.0)

        r2_sb = vp.tile([P, 8, B_LOCAL], bf16, tag="r2v", bufs=1)
        e2_sb = vp.tile([P, 8, B_LOCAL], bf16, tag="e2v", bufs=1)
        for mt in range(8):
            ps = psum.tile([P, B_LOCAL], f32, tag="ps")
            n_acc = 16
            i = 0
            for kt in range(8):
                for h1v in (r1_sb, e1_sb):
                    nc.tensor.matmul(
                        ps[:],
                        w2_sb[:, kt, mt * P : (mt + 1) * P],
                        h1v[:, kt, :],
                        start=(i == 0),
                        stop=(i == n_acc - 1),
                    )
                    i += 1
            nc.scalar.activation(
                r2_sb[:, mt, :], ps[:], AF.Relu, bias=b2_sb[:, mt : mt + 1]
            )
            u = vp.tile([P, B_LOCAL], f32, tag="elu_u")
            nc.scalar.activation(
                u[:], ps[:], AF.Relu, bias=nb2_sb[:, mt : mt + 1], scale=-1.0
            )
            nc.scalar.activation(e2_sb[:, mt, :], u[:], AF.Exp, scale=-1.0)

        for g in range(NBLK):
            pv = psv.tile([P, 1], f32, tag="pv")
            n_acc = 16
            i = 0
            for kt in range(8):
                for h2v in (r2_sb, e2_sb):
                    nc.tensor.matmul(
                        pv[:],
                        h2v[:, kt, g * P : (g + 1) * P],
                        w3_sb[:, kt : kt + 1],
                        start=(i == 0),
                        stop=(i == n_acc - 1),
                    )
                    i += 1
            nc.scalar.activation(
                out_tiles[g][:, 0:1], pv[:], AF.Identity, bias=b3r_sb[:, 0:1]
            )

        for g in range(NBLK):
            rows = slice(g * P, (g + 1) * P)
            nc.scalar.dma_start(out_d[rows, :], out_tiles[g][:])

    nc.compile()
    return nc


def _get_nc():
    if "nc" not in _NC_CACHE:
        _NC_CACHE["nc"] = _build_nc()
    return _NC_CACHE["nc"]


def _prep_in_maps(inputs):
    x = np.ascontiguousarray(np.asarray(inputs["inputs"], dtype=np.float32))
    opt = np.asarray(inputs["option"], dtype=np.float32)
    w1 = np.asarray(inputs["w1"], dtype=np.float32)
    b1 = np.asarray(inputs["b1"], dtype=np.float32)
    w2 = np.asarray(inputs["w2"], dtype=np.float32)
    b2 = np.asarray(inputs["b2"], dtype=np.float32)
    w3 = np.asarray(inputs["w3"], dtype=np.float32)
    b3 = np.asarray(inputs["b3"], dtype=np.float32)

    opt_bf = _bf16(opt[:, _option_perm()])

    # weight [K, M] with K across 128-partition tiles -> [128, n_k_tiles * M]
    def ktiled(a):  # a: [K, M]
        k, m = a.shape
        return a.reshape(k // P, P, m).transpose(1, 0, 2).reshape(P, -1)

    w1t = ktiled(w1.T)  # [128, 2*1024]
    w2t = ktiled(w2.T)  # [128, 8*1024]
    w3t = w3.reshape(8, P).T  # [128, 8]
    b1t = b1.reshape(8, P).T  # [128, 8]
    # device computes elu+1 (= relu(z)+exp(min(z,0))); fold the -1 into the
    # consumer's bias: b' = b - W.sum(axis=1)
    b2p = b2 - w2.sum(axis=1)
    b2t = b2p.reshape(8, P).T
    b3p = float(b3[0] - w3.sum())
    b3r = np.full((P, 1), b3p, dtype=np.float32)
    wtail = np.concatenate([w1t, w2t, w3t], axis=1)
    bpack = np.ascontiguousarray(
        np.concatenate([b1t, -b1t, b2t, -b2t, b3r], axis=1), dtype=np.float32
    )
    assert bpack.shape == (P, BPACK_W)

    in_maps = []
    for c in range(NCORES):
        sl = slice(c * B_LOCAL, (c + 1) * B_LOCAL)
        xs = np.ascontiguousarray(x[sl])
        xt = ktiled(xs.T)  # [128, 2*512]
        wpack = np.ascontiguousarray(_bf16(np.concatenate([xt, wtail], axis=1)))
        assert wpack.shape == (P, WPACK_W)
        in_maps.append(
            {
                "option": np.ascontiguousarray(opt_bf[sl]),
                "inputs": xs,
                "wpack": wpack,
                "bpack": bpack,
            }
        )
    return in_maps


def _ensure_ntff_hook():
    """Provide antenv.axon_hooks (missing in this image) so trace=True works."""
    import sys
    import types

    if "antenv.axon_hooks" in sys.modules:
        return
    mod = types.ModuleType("antenv.axon_hooks")
    state = {"hook": None}
    mod.set_axon_ntff_profile_hook = lambda h: state.__setitem__("hook", h)
    mod.get_axon_ntff_profile_hook = lambda: state["hook"]
    sys.modules["antenv.axon_hooks"] = mod
    import antenv

    antenv.axon_hooks = mod
    try:
        from trn_agent_boot.trn_boot import _ntff_profile_via_ctypes

        hook = _ntff_profile_via_ctypes("/opt/axon/libaxon_pjrt.so")
        mod.set_axon_ntff_profile_hook(hook)
    except Exception as e:  # degrade: tracing skipped, run still works
        print(f"ntff hook setup failed: {e}")


def run(inputs, trace=False):
    """Returns (full_output [4096, 33] float32, exec_time_ns or None)."""
    from concourse.bass_utils import run_bass_kernel_spmd

    if trace:
        _ensure_ntff_hook()
    nc = _get_nc()
    in_maps = _prep_in_maps(inputs)
    res = run_bass_kernel_spmd(nc, in_maps, core_ids=list(range(NCORES)), trace=trace)
    out = np.concatenate([res.results[i]["out"] for i in range(NCORES)], axis=0)
    return out.astype(np.float32), res.exec_time_ns


def kernel(**inputs):
    out, _ = run(inputs, trace=False)
    return out
